# revision 42
# baseline (speedup 1.0000x reference)
"""Trainium2 Bass kernel for nn_DeconvBlock (dynamic-weight transposed conv).

Computes, per sample b:
    w_b   = weight + sum_j feature[b,j] * (t_j * m_j)            (weight synthesis)
    out_b = conv_transpose2d(x_b, w_b, stride=2, pad=1, K=4)     (grouped over batch)
    out   = prelu(out_b + bias, a)

Strategy (data-parallel over batch, 8 cores x 2 samples):
  - conv_transpose(stride 2, K=4, P=1) decomposes into 4 output phases
    (py,px) in {0,1}^2; each phase output pixel is a sum of 4 "taps"
    (ky,kx), each tap a 1x1 conv (matmul over CIN=256) of a +-1 shifted x.
  - Operands are fp16 (PE streams 16-bit moving operands at 1 col/cycle;
    accumulation stays fp32 in PSUM; measured rel err ~2.4e-4). 512+
    matmuls/core of [128x128] @ [128x512] ~= 112us = the PE streaming
    roofline for the 4.3 GMAC/core workload; the kernel keeps that stream
    gap-free and minimizes time outside it.
  - Per-sample weight synthesis (0.2% of the FLOPs) happens on the host
    as part of input layout prep: one (B,4)x(4,CIN*COUT*K*K) sgemm.
  - DMA cost on TRN2 is dominated by per-partition descriptor generation
    (~one descriptor per partition per contiguous run), so inputs are
    host-packed into few large-descriptor transfers: the first block's
    entire working set (phase-0 weights + x rows 0:10 of both chunks)
    arrives in two parallel DMAs, one per HWDGE queue (sync + scalar).
  - Scratch warm-up matmuls run during the DMA wait so the PE HAM clock
    gate is already at 2.4 GHz when the real stream starts.
  - Epilogue: one fused ScalarE op per phase computes prelu(ps + bias)
    (Prelu activation with per-partition bias and slope) straight from
    PSUM while interleaving the 4 phases into contiguous rows; each row
    block then leaves in ONE contiguous DMA (8KB/partition descriptors),
    alternating between the two HWDGE queues; the final rows are split
    into small blocks whose DMAs drain in parallel on both queues.
"""

import numpy as np

import concourse.mybir as mybir
from concourse import bacc
from concourse import bass_utils
from concourse.tile import TileContext

B, CIN, COUT, H, W, K, S = 16, 256, 128, 64, 64, 4, 2
NCORES = 8
BPC = B // NCORES  # samples per core
P = 128
NCH = CIN // P     # ic chunks of 128
HP = H + 2         # padded x height/width (zero border of 1)
NROW = 8           # output-phase rows per block
NYB = H // NROW    # row blocks per sample
NWARM = 34         # PE warm-up matmuls bridging the startup DMA wait
XR0 = 10           # x rows packed into the startup tensors
WPH = NCH * 2 * 2 * COUT  # weight elems per phase (1024)

# phase py -> ((ky, sy), ...): contribution x[y'+sy] * w[ky]
_TAPS = {0: ((1, 0), (3, -1)), 1: ((2, 0), (0, 1))}

_COMPILED = None


def _build():
    f32 = mybir.dt.float32
    f16 = mybir.dt.float16
    Act = mybir.ActivationFunctionType

    nc = bacc.Bacc(
        "TRN2", target_bir_lowering=False, debug=False, num_devices=NCORES
    )
    x_d = nc.dram_tensor(
        "x_sh", (BPC, NCH, P, HP, HP), f16, kind="ExternalInput"
    ).ap()
    # startup packs (sample 0): su0 = [w_ph0 | w_ph1], su1 = [x c0 rows
    # 0:10 | x c1 rows 0:10], su2 = [w_ph2 | w_ph3]
    su0_d = nc.dram_tensor("su0", (P, 2 * WPH), f16, kind="ExternalInput").ap()
    su1_d = nc.dram_tensor(
        "su1", (P, 2 * XR0 * HP), f16, kind="ExternalInput"
    ).ap()
    su2_d = nc.dram_tensor("su2", (P, 2 * WPH), f16, kind="ExternalInput").ap()
    # sample-1 weights, phase-grouped: (cin_part, phase, chunk, iy, ix, cout)
    w1_d = nc.dram_tensor(
        "w1", (P, 4, NCH, 2, 2, COUT), f16, kind="ExternalInput"
    ).ap()
    ba_d = nc.dram_tensor("ba", (P, 2), f32, kind="ExternalInput").ap()
    out_d = nc.dram_tensor(
        "out_sh", (BPC, COUT, H * S, W * S), f32, kind="ExternalOutput"
    ).ap()

    with TileContext(nc) as tc:
        with (
            tc.tile_pool(name="const", bufs=1) as const_pool,
            tc.tile_pool(name="w_pool", bufs=1) as w_pool,
            tc.tile_pool(name="x_pool", bufs=1) as x_pool,
            tc.tile_pool(name="row_pool", bufs=6) as row_pool,
            tc.tile_pool(name="psum", bufs=4, space="PSUM") as psum_pool,
        ):
            ba_t = const_pool.tile([P, 2], f32)
            bias_t = ba_t[:, 0:1]
            a_t = ba_t[:, 1:2]
            warm_t = const_pool.tile([P, P], f16)
            scratch_t = const_pool.tile([P, 1], f32)

            su0_t = w_pool.tile([P, 2 * WPH], f16, name="su0t", tag="su0t")
            su1_t = w_pool.tile(
                [P, 2 * XR0 * HP], f16, name="su1t", tag="su1t"
            )
            su2_t = w_pool.tile([P, 2 * WPH], f16, name="su2t", tag="su2t")
            wt1 = w_pool.tile(
                [P, 4, NCH, 2, 2, COUT], f16, name="wt1", tag="wt1"
            )
            xt = []
            for s in range(BPC):
                xt.append(
                    x_pool.tile(
                        [P, NCH, HP, HP], f16, name=f"xpad{s}", tag=f"xpad{s}"
                    )
                )

            # x rows 0:10 views inside the startup pack (per chunk)
            surow = [
                su1_t[:, 0 : XR0 * HP].rearrange(
                    "p (r c) -> p r c", r=XR0, c=HP
                ),
                su1_t[:, XR0 * HP :].rearrange(
                    "p (r c) -> p r c", r=XR0, c=HP
                ),
            ]

            def lhsT(s, ph, c, iy, ix):
                if s == 1:
                    return wt1[:, ph, c, iy, ix, :]
                off = ((c * 2 + iy) * 2 + ix) * COUT
                if ph < 2:
                    base = ph * WPH + off
                    return su0_t[:, base : base + COUT]
                base = (ph - 2) * WPH + off
                return su2_t[:, base : base + COUT]

            # ---- startup DMAs ----
            # The first block's working set lands as small parallel
            # transfers, one per HWDGE queue, ahead of everything else.
            # The larger x pack rides the sync queue (DGE start 650ns vs
            # scalar's 784ns) so the two critical chains finish together.
            nc.sync.dma_start(su1_t[:], su1_d[:])
            nc.scalar.dma_start(su0_t[:, 0:WPH], su0_d[:, 0:WPH])
            nc.sync.dma_start(su0_t[:, WPH:], su0_d[:, WPH:])
            nc.sync.dma_start(su2_t[:], su2_d[:])
            nc.scalar.dma_start(xt[0][:, 0, 0:26], x_d[0, 0, :, 0:26])
            nc.scalar.dma_start(xt[0][:, 1, 0:26], x_d[0, 1, :, 0:26])
            nc.sync.dma_start(ba_t[:], ba_d[:])
            nc.sync.dma_start(wt1[:], w1_d[:])
            nc.scalar.dma_start(xt[0][:, 0, 26:HP], x_d[0, 0, :, 26:HP])
            nc.scalar.dma_start(xt[0][:, 1, 26:HP], x_d[0, 1, :, 26:HP])
            nc.scalar.dma_start(xt[1][:, 0], x_d[1, 0])
            nc.scalar.dma_start(xt[1][:, 1], x_d[1, 1])

            # warm the ScalarE activation table (Prelu) during DMA wait
            nc.vector.memset(scratch_t[:], 0.0)
            nc.scalar.activation(
                scratch_t[:], scratch_t[:], Act.Prelu, scale=1.0, alpha=0.25
            )

            # ---- PE warm-up: scratch matmuls while inputs stream in ----
            nc.vector.memset(warm_t[:], 0.0)
            for i in range(NWARM):
                ps_w = psum_pool.tile([P, 2, NROW, W], f32, name="ps", tag="ps")
                nc.tensor.matmul(
                    ps_w[:, 0, 0:2, :], warm_t[:], warm_t[:], start=True, stop=True
                )

            # ---- main conv loop ----
            # The final sample's last rows are split into small blocks so
            # the post-stream drain (bias+prelu+DMA of the last block) is
            # short, with DMAs in parallel on both HWDGE queues.
            blocks = [(NROW * i, NROW) for i in range(NYB)]
            last_blocks = blocks[:-1] + [
                (NROW * (NYB - 1), 4),
                (NROW * (NYB - 1) + 4, 2),
                (NROW * (NYB - 1) + 6, 2),
            ]
            for s in range(BPC):
                for by0, nr in last_blocks if s == BPC - 1 else blocks:
                    row_t = row_pool.tile(
                        [P, nr, 2, W, 2], f32, name="row_t", tag="row_t"
                    )
                    for py in (0, 1):
                        # both px phases accumulate into bank-disjoint
                        # halves of one 2-bank PSUM tile, drained by a
                        # single fused ScalarE prelu(ps + bias) op
                        ps = psum_pool.tile(
                            [P, 2, nr, W], f32, name="ps", tag="ps"
                        )
                        for px in (0, 1):
                            ph = 2 * py + px
                            k = 0
                            for c in range(NCH):
                                for iy, (ky, sy) in enumerate(_TAPS[py]):
                                    for ix, (kx, sx) in enumerate(_TAPS[px]):
                                        y0 = 1 + sy + by0
                                        x0 = 1 + sx
                                        if s == 0 and by0 == 0:
                                            rhs = surow[c][
                                                :, y0 : y0 + nr, x0 : x0 + W
                                            ]
                                        else:
                                            rhs = xt[s][
                                                :, c, y0 : y0 + nr, x0 : x0 + W
                                            ]
                                        nc.tensor.matmul(
                                            ps[:, px],
                                            lhsT(s, ph, c, iy, ix),
                                            rhs,
                                            start=(k == 0),
                                            stop=(k == 7),
                                        )
                                        k += 1
                        nc.scalar.activation(
                            row_t[:, :, py, :, :].rearrange(
                                "p y x q -> p q y x"
                            ),
                            ps[:],
                            Act.Prelu,
                            bias=bias_t,
                            scale=1.0,
                            alpha=a_t,
                        )
                    # one contiguous DMA per block: rows 2*by0 .. 2*by0+2*nr
                    last = s == BPC - 1 and by0 + nr == H
                    if last:
                        h = nr // 2
                        nc.sync.dma_start(
                            out_d[s, :, 2 * by0 : 2 * (by0 + h), :],
                            row_t[:, 0:h],
                        )
                        nc.scalar.dma_start(
                            out_d[s, :, 2 * (by0 + h) : 2 * (by0 + nr), :],
                            row_t[:, h:nr],
                        )
                    elif (by0 // NROW) % 2 == 0:
                        nc.sync.dma_start(
                            out_d[s, :, 2 * by0 : 2 * (by0 + nr), :],
                            row_t[:],
                        )
                    else:
                        nc.scalar.dma_start(
                            out_d[s, :, 2 * by0 : 2 * (by0 + nr), :],
                            row_t[:],
                        )

    nc.compile()
    return nc


def _get_compiled():
    global _COMPILED
    if _COMPILED is None:
        _COMPILED = _build()
    return _COMPILED


# host-side tap gather indices: KY[ph,iy,ix], KX[ph,iy,ix]
_KG = np.array([[1, 3], [2, 0]])  # [p, i] -> k index
_KY = np.zeros((4, 2, 2), np.intp)
_KX = np.zeros((4, 2, 2), np.intp)
for _py in range(2):
    for _px in range(2):
        for _iy in range(2):
            for _ix in range(2):
                _KY[2 * _py + _px, _iy, _ix] = _KG[_py, _iy]
                _KX[2 * _py + _px, _iy, _ix] = _KG[_px, _ix]


def _prep_in_maps(inputs):
    x = np.asarray(inputs["x"], dtype=np.float32)
    xp = np.zeros((B, NCH, P, HP, HP), dtype=np.float16)
    xp[:, :, :, 1 : HP - 1, 1 : HP - 1] = x.reshape(B, NCH, P, H, W)

    # per-sample weight synthesis: one (B,4) @ (4, CIN*COUT*K*K) sgemm
    feat = np.asarray(inputs["feature"], dtype=np.float32)
    w = np.asarray(inputs["weight"], dtype=np.float32)
    tm = np.stack(
        [
            np.asarray(inputs[f"t_{n}"], dtype=np.float32)[0]
            * np.asarray(inputs[f"m_{n}"], dtype=np.float32)[0]
            for n in ("bayer", "quad", "nano", "qxq")
        ]
    )  # (4, CIN, COUT, K, K)
    wb = (feat @ tm.reshape(4, -1)).reshape(B, CIN, COUT, K, K)
    wb += w[None]
    # phase-grouped gather: -> (P, B, 4, NCH*2*2*COUT) fp16
    wr = wb.reshape(B, NCH, P, COUT, K, K)
    wsel = wr[:, :, :, :, _KY, _KX]  # (B, NCH, P, COUT, 4, 2, 2)
    wph = np.ascontiguousarray(
        wsel.transpose(2, 0, 4, 1, 5, 6, 3), dtype=np.float16
    ).reshape(P, B, 4, WPH)

    bias = np.asarray(inputs["bias"], dtype=np.float32).reshape(P, 1)
    a = np.broadcast_to(
        np.asarray(inputs["prelu_a"], dtype=np.float32).reshape(1, 1), (P, 1)
    )
    ba = np.ascontiguousarray(np.concatenate([bias, a], axis=1))

    in_maps = []
    for i in range(NCORES):
        s0 = i * BPC
        xr0 = xp[s0, 0, :, 0:XR0].reshape(P, -1)  # (P, 660)
        xr1 = xp[s0, 1, :, 0:XR0].reshape(P, -1)
        su0 = np.ascontiguousarray(
            np.concatenate([wph[:, s0, 0], wph[:, s0, 1]], axis=1)
        )
        su1 = np.ascontiguousarray(np.concatenate([xr0, xr1], axis=1))
        su2 = np.ascontiguousarray(
            np.concatenate([wph[:, s0, 2], wph[:, s0, 3]], axis=1)
        )
        in_maps.append(
            {
                "x_sh": xp[s0 : s0 + BPC],
                "su0": su0,
                "su1": su1,
                "su2": su2,
                "w1": np.ascontiguousarray(wph[:, s0 + 1]).reshape(
                    P, 4, NCH, 2, 2, COUT
                ),
                "ba": ba,
            }
        )
    return in_maps


def kernel(**inputs):
    nc = _get_compiled()
    in_maps = _prep_in_maps(inputs)
    res = bass_utils.run_bass_kernel_spmd(nc, in_maps, core_ids=list(range(NCORES)))
    return np.concatenate(
        [res.results[i]["out_sh"] for i in range(NCORES)], axis=0
    )


# revision 43
# speedup vs baseline: 1.0186x; 1.0186x over previous
"""Trainium2 Bass kernel for nn_DeconvBlock (dynamic-weight transposed conv).

Computes, per sample b:
    w_b   = weight + sum_j feature[b,j] * (t_j * m_j)            (weight synthesis)
    out_b = conv_transpose2d(x_b, w_b, stride=2, pad=1, K=4)     (grouped over batch)
    out   = prelu(out_b + bias, a)

Strategy (data-parallel over batch, 8 cores x 2 samples):
  - conv_transpose(stride 2, K=4, P=1) decomposes into 4 output phases
    (py,px) in {0,1}^2; each phase output pixel is a sum of 4 "taps"
    (ky,kx), each tap a 1x1 conv (matmul over CIN=256) of a +-1 shifted x.
  - Operands are fp16 (PE streams 16-bit moving operands at 1 col/cycle;
    accumulation stays fp32 in PSUM; measured rel err ~2.4e-4). 512+
    matmuls/core of [128x128] @ [128x512] ~= 112us = the PE streaming
    roofline for the 4.3 GMAC/core workload; the kernel keeps that stream
    gap-free and minimizes time outside it.
  - Per-sample weight synthesis (0.2% of the FLOPs) happens on the host
    as part of input layout prep: one (B,4)x(4,CIN*COUT*K*K) sgemm.
  - DMA cost on TRN2 is dominated by per-partition descriptor generation
    (~one descriptor per partition per contiguous run), so inputs are
    host-packed into few large-descriptor transfers: the first block's
    entire working set (phase-0 weights + x rows 0:10 of both chunks)
    arrives in two parallel DMAs, one per HWDGE queue (sync + scalar).
  - Scratch warm-up matmuls run during the DMA wait so the PE HAM clock
    gate is already at 2.4 GHz when the real stream starts.
  - Epilogue: one fused ScalarE op per phase computes prelu(ps + bias)
    (Prelu activation with per-partition bias and slope) straight from
    PSUM while interleaving the 4 phases into contiguous rows; each row
    block then leaves in ONE contiguous DMA (8KB/partition descriptors),
    alternating between the two HWDGE queues; the final rows are split
    into small blocks whose DMAs drain in parallel on both queues.
"""

import numpy as np

import concourse.mybir as mybir
from concourse import bacc
from concourse import bass_utils
from concourse.tile import TileContext

B, CIN, COUT, H, W, K, S = 16, 256, 128, 64, 64, 4, 2
NCORES = 8
BPC = B // NCORES  # samples per core
P = 128
NCH = CIN // P     # ic chunks of 128
HP = H + 2         # padded x height/width (zero border of 1)
NROW = 8           # output-phase rows per block
NYB = H // NROW    # row blocks per sample
NWARM = 30         # PE warm-up matmuls bridging the startup DMA wait
XR0 = 10           # x rows packed into the startup tensors
WPH = NCH * 2 * 2 * COUT  # weight elems per phase (1024)

# phase py -> ((ky, sy), ...): contribution x[y'+sy] * w[ky]
_TAPS = {0: ((1, 0), (3, -1)), 1: ((2, 0), (0, 1))}

_COMPILED = None


def _build():
    f32 = mybir.dt.float32
    f16 = mybir.dt.float16
    Act = mybir.ActivationFunctionType

    nc = bacc.Bacc(
        "TRN2", target_bir_lowering=False, debug=False, num_devices=NCORES
    )
    x_d = nc.dram_tensor(
        "x_sh", (BPC, NCH, P, HP, HP), f16, kind="ExternalInput"
    ).ap()
    # startup packs (sample 0): su0 = [w_ph0 | w_ph1], su1 = [x c0 rows
    # 0:10 | x c1 rows 0:10], su2 = [w_ph2 | w_ph3]
    su0_d = nc.dram_tensor("su0", (P, 2 * WPH), f16, kind="ExternalInput").ap()
    su1_d = nc.dram_tensor(
        "su1", (P, 2 * XR0 * HP), f16, kind="ExternalInput"
    ).ap()
    su2_d = nc.dram_tensor("su2", (P, 2 * WPH), f16, kind="ExternalInput").ap()
    # sample-1 weights, phase-grouped: (cin_part, phase, chunk, iy, ix, cout)
    w1_d = nc.dram_tensor(
        "w1", (P, 4, NCH, 2, 2, COUT), f16, kind="ExternalInput"
    ).ap()
    ba_d = nc.dram_tensor("ba", (P, 2), f32, kind="ExternalInput").ap()
    out_d = nc.dram_tensor(
        "out_sh", (BPC, COUT, H * S, W * S), f32, kind="ExternalOutput"
    ).ap()

    with TileContext(nc) as tc:
        with (
            tc.tile_pool(name="const", bufs=1) as const_pool,
            tc.tile_pool(name="w_pool", bufs=1) as w_pool,
            tc.tile_pool(name="x_pool", bufs=1) as x_pool,
            tc.tile_pool(name="row_pool", bufs=6) as row_pool,
            tc.tile_pool(name="psum", bufs=4, space="PSUM") as psum_pool,
        ):
            ba_t = const_pool.tile([P, 2], f32)
            bias_t = ba_t[:, 0:1]
            a_t = ba_t[:, 1:2]
            warm_t = const_pool.tile([P, P], f16)
            scratch_t = const_pool.tile([P, 1], f32)

            su0_t = w_pool.tile([P, 2 * WPH], f16, name="su0t", tag="su0t")
            su1_t = w_pool.tile(
                [P, 2 * XR0 * HP], f16, name="su1t", tag="su1t"
            )
            su2_t = w_pool.tile([P, 2 * WPH], f16, name="su2t", tag="su2t")
            wt1 = w_pool.tile(
                [P, 4, NCH, 2, 2, COUT], f16, name="wt1", tag="wt1"
            )
            xt = []
            for s in range(BPC):
                xt.append(
                    x_pool.tile(
                        [P, NCH, HP, HP], f16, name=f"xpad{s}", tag=f"xpad{s}"
                    )
                )

            # x rows 0:10 views inside the startup pack (per chunk)
            surow = [
                su1_t[:, 0 : XR0 * HP].rearrange(
                    "p (r c) -> p r c", r=XR0, c=HP
                ),
                su1_t[:, XR0 * HP :].rearrange(
                    "p (r c) -> p r c", r=XR0, c=HP
                ),
            ]

            def lhsT(s, ph, c, iy, ix):
                if s == 1:
                    return wt1[:, ph, c, iy, ix, :]
                off = ((c * 2 + iy) * 2 + ix) * COUT
                if ph < 2:
                    base = ph * WPH + off
                    return su0_t[:, base : base + COUT]
                base = (ph - 2) * WPH + off
                return su2_t[:, base : base + COUT]

            # ---- startup DMAs ----
            # The first block's working set lands as small parallel
            # transfers, one per HWDGE queue, ahead of everything else.
            # The larger x pack rides the sync queue (DGE start 650ns vs
            # scalar's 784ns) so the two critical chains finish together.
            nc.sync.dma_start(su1_t[:], su1_d[:])
            nc.scalar.dma_start(su0_t[:, 0:WPH], su0_d[:, 0:WPH])
            nc.sync.dma_start(su0_t[:, WPH:], su0_d[:, WPH:])
            nc.sync.dma_start(su2_t[:], su2_d[:])
            nc.scalar.dma_start(xt[0][:, 0, 0:26], x_d[0, 0, :, 0:26])
            nc.scalar.dma_start(xt[0][:, 1, 0:26], x_d[0, 1, :, 0:26])
            nc.sync.dma_start(ba_t[:], ba_d[:])
            nc.sync.dma_start(wt1[:], w1_d[:])
            nc.scalar.dma_start(xt[0][:, 0, 26:HP], x_d[0, 0, :, 26:HP])
            nc.scalar.dma_start(xt[0][:, 1, 26:HP], x_d[0, 1, :, 26:HP])
            nc.scalar.dma_start(xt[1][:, 0], x_d[1, 0])
            nc.scalar.dma_start(xt[1][:, 1], x_d[1, 1])

            # warm the ScalarE activation table (Prelu) during DMA wait
            nc.vector.memset(scratch_t[:], 0.0)
            nc.scalar.activation(
                scratch_t[:], scratch_t[:], Act.Prelu, scale=1.0, alpha=0.25
            )

            # ---- PE warm-up: scratch matmuls while inputs stream in ----
            nc.vector.memset(warm_t[:], 0.0)
            for i in range(NWARM):
                ps_w = psum_pool.tile([P, 2, NROW, W], f32, name="ps", tag="ps")
                nc.tensor.matmul(
                    ps_w[:, 0, 0:2, :], warm_t[:], warm_t[:], start=True, stop=True
                )

            # ---- main conv loop ----
            # The final sample's last rows are split into small blocks so
            # the post-stream drain (bias+prelu+DMA of the last block) is
            # short, with DMAs in parallel on both HWDGE queues.
            blocks = [(NROW * i, NROW) for i in range(NYB)]
            last_blocks = blocks[:-1] + [
                (NROW * (NYB - 1), 4),
                (NROW * (NYB - 1) + 4, 2),
                (NROW * (NYB - 1) + 6, 2),
            ]
            for s in range(BPC):
                for by0, nr in last_blocks if s == BPC - 1 else blocks:
                    row_t = row_pool.tile(
                        [P, nr, 2, W, 2], f32, name="row_t", tag="row_t"
                    )
                    for py in (0, 1):
                        # both px phases accumulate into bank-disjoint
                        # halves of one 2-bank PSUM tile, drained by a
                        # single fused ScalarE prelu(ps + bias) op
                        ps = psum_pool.tile(
                            [P, 2, nr, W], f32, name="ps", tag="ps"
                        )
                        for px in (0, 1):
                            ph = 2 * py + px
                            k = 0
                            for c in range(NCH):
                                for iy, (ky, sy) in enumerate(_TAPS[py]):
                                    for ix, (kx, sx) in enumerate(_TAPS[px]):
                                        y0 = 1 + sy + by0
                                        x0 = 1 + sx
                                        if s == 0 and by0 == 0:
                                            rhs = surow[c][
                                                :, y0 : y0 + nr, x0 : x0 + W
                                            ]
                                        else:
                                            rhs = xt[s][
                                                :, c, y0 : y0 + nr, x0 : x0 + W
                                            ]
                                        nc.tensor.matmul(
                                            ps[:, px],
                                            lhsT(s, ph, c, iy, ix),
                                            rhs,
                                            start=(k == 0),
                                            stop=(k == 7),
                                        )
                                        k += 1
                        nc.scalar.activation(
                            row_t[:, :, py, :, :].rearrange(
                                "p y x q -> p q y x"
                            ),
                            ps[:],
                            Act.Prelu,
                            bias=bias_t,
                            scale=1.0,
                            alpha=a_t,
                        )
                    # one contiguous DMA per block: rows 2*by0 .. 2*by0+2*nr
                    last = s == BPC - 1 and by0 + nr == H
                    if last:
                        h = nr // 2
                        nc.sync.dma_start(
                            out_d[s, :, 2 * by0 : 2 * (by0 + h), :],
                            row_t[:, 0:h],
                        )
                        nc.scalar.dma_start(
                            out_d[s, :, 2 * (by0 + h) : 2 * (by0 + nr), :],
                            row_t[:, h:nr],
                        )
                    elif (by0 // NROW) % 2 == 0:
                        nc.sync.dma_start(
                            out_d[s, :, 2 * by0 : 2 * (by0 + nr), :],
                            row_t[:],
                        )
                    else:
                        nc.scalar.dma_start(
                            out_d[s, :, 2 * by0 : 2 * (by0 + nr), :],
                            row_t[:],
                        )

    nc.compile()
    return nc


def _get_compiled():
    global _COMPILED
    if _COMPILED is None:
        _COMPILED = _build()
    return _COMPILED


# host-side tap gather indices: KY[ph,iy,ix], KX[ph,iy,ix]
_KG = np.array([[1, 3], [2, 0]])  # [p, i] -> k index
_KY = np.zeros((4, 2, 2), np.intp)
_KX = np.zeros((4, 2, 2), np.intp)
for _py in range(2):
    for _px in range(2):
        for _iy in range(2):
            for _ix in range(2):
                _KY[2 * _py + _px, _iy, _ix] = _KG[_py, _iy]
                _KX[2 * _py + _px, _iy, _ix] = _KG[_px, _ix]


def _prep_in_maps(inputs):
    x = np.asarray(inputs["x"], dtype=np.float32)
    xp = np.zeros((B, NCH, P, HP, HP), dtype=np.float16)
    xp[:, :, :, 1 : HP - 1, 1 : HP - 1] = x.reshape(B, NCH, P, H, W)

    # per-sample weight synthesis: one (B,4) @ (4, CIN*COUT*K*K) sgemm
    feat = np.asarray(inputs["feature"], dtype=np.float32)
    w = np.asarray(inputs["weight"], dtype=np.float32)
    tm = np.stack(
        [
            np.asarray(inputs[f"t_{n}"], dtype=np.float32)[0]
            * np.asarray(inputs[f"m_{n}"], dtype=np.float32)[0]
            for n in ("bayer", "quad", "nano", "qxq")
        ]
    )  # (4, CIN, COUT, K, K)
    wb = (feat @ tm.reshape(4, -1)).reshape(B, CIN, COUT, K, K)
    wb += w[None]
    # phase-grouped gather: -> (P, B, 4, NCH*2*2*COUT) fp16
    wr = wb.reshape(B, NCH, P, COUT, K, K)
    wsel = wr[:, :, :, :, _KY, _KX]  # (B, NCH, P, COUT, 4, 2, 2)
    wph = np.ascontiguousarray(
        wsel.transpose(2, 0, 4, 1, 5, 6, 3), dtype=np.float16
    ).reshape(P, B, 4, WPH)

    bias = np.asarray(inputs["bias"], dtype=np.float32).reshape(P, 1)
    a = np.broadcast_to(
        np.asarray(inputs["prelu_a"], dtype=np.float32).reshape(1, 1), (P, 1)
    )
    ba = np.ascontiguousarray(np.concatenate([bias, a], axis=1))

    in_maps = []
    for i in range(NCORES):
        s0 = i * BPC
        xr0 = xp[s0, 0, :, 0:XR0].reshape(P, -1)  # (P, 660)
        xr1 = xp[s0, 1, :, 0:XR0].reshape(P, -1)
        su0 = np.ascontiguousarray(
            np.concatenate([wph[:, s0, 0], wph[:, s0, 1]], axis=1)
        )
        su1 = np.ascontiguousarray(np.concatenate([xr0, xr1], axis=1))
        su2 = np.ascontiguousarray(
            np.concatenate([wph[:, s0, 2], wph[:, s0, 3]], axis=1)
        )
        in_maps.append(
            {
                "x_sh": xp[s0 : s0 + BPC],
                "su0": su0,
                "su1": su1,
                "su2": su2,
                "w1": np.ascontiguousarray(wph[:, s0 + 1]).reshape(
                    P, 4, NCH, 2, 2, COUT
                ),
                "ba": ba,
            }
        )
    return in_maps


def kernel(**inputs):
    nc = _get_compiled()
    in_maps = _prep_in_maps(inputs)
    res = bass_utils.run_bass_kernel_spmd(nc, in_maps, core_ids=list(range(NCORES)))
    return np.concatenate(
        [res.results[i]["out_sh"] for i in range(NCORES)], axis=0
    )
